# revision 11
# baseline (speedup 1.0000x reference)
"""Bahdanau-style attention scoring kernel for 8 TRN2 NeuronCores.

Reference computation (B=128, H=256, N=2048):
    hidden = concat([static, dynamic, broadcast(dec)], axis=1)   # [B, 3H, N]
    scores = tanh(einsum('hk,bkn->bhn', W[0], hidden))           # [B, H, N]
    logits = einsum('h,bhn->bn', v[0,0], scores)                 # [B, N]
    attns  = softmax(logits, axis=-1)[:, None, :]                # [B, 1, N]

Strategy v2 (v1 measured ~210-230us; this version targets ~175us):

- Data-parallel over batch: 16 batches per core, no collectives. The tiny
  W / v params are replicated (pre-cast to bf16 on host); the broadcast
  decoder term collapses to a per-batch bias c[b] = W_dec @ dec[b]
  (precomputed on host, 0.003% of FLOPs).

- Matmuls run in bf16 at ~222ns per 512-col tile (near the 2.4GHz PE
  roofline). f32 activations are DMA'd raw; the matmul rhs reads the high
  2 bytes of each f32 in SBUF (stride-2 bf16 bitcast view) -- bf16
  truncation for free. PE work per batch ~8.9us (32 mains + 8 v-matmuls);
  DMA per batch 4MB ~9.5us at the measured ~420GB/s => DMA-bound steady
  state with the PE never the gating leg.

- v1's trace showed a 13.5us/batch period: the PE stalled ~3.5us/batch on
  a tanh->PSUM-slot WAR chain (2x2-bank pst slots) plus ~1us on late x
  tiles. v2 uses 1-bank PSUM accumulators with 6 slots (tag "pst"), so
  slot reuse lands ~6 tanh-units back and the chain has ~4us of slack.

- All 64 logits land in ONE PSUM bank: the masked v-matmul for n-tile nt
  writes output partitions [32*nt, 32*nt+32) (tile_position col offsets
  must be multiples of 32), with vm columns 16..31 zero so the unused
  partitions accumulate exact zeros instead of stale PSUM garbage.
  Softmax is then: one exp over [128,512] with accum_out row sums, a tiny
  f32 mask-matmul that sums each batch's 4 quarter-sums across partitions
  (placing the total on all 4 of that batch's partitions), reciprocal,
  one [128,512] scale, 4 output DMAs.

- x loads: one 2MB HWDGE DMA per (batch, source) (split into column
  chunks for batches 0/1 so the first matmuls start ~4us in), prefetched
  5 batches deep. Parameter DMAs go on the idle GPSIMD (SWDGE) ring so
  the first x tile is not queued behind them.

Built as a bacc.Bacc graph (its compile() pass redistributes multi-sem
waits; raw Bass hits the hardware's one-sync-wait-per-instruction limit).
"""

import sys

if "/opt/trn_rl_repo" not in sys.path:
    sys.path.insert(0, "/opt/trn_rl_repo")

import numpy as np

B, H, N = 128, 256, 2048
NCORES = 8
BPC = B // NCORES  # batches per core
P = 128            # SBUF partitions
KT = 4             # k-tiles over 2H=512 contraction
MT = 2             # m-tiles over H=256 output rows
NS = 512           # n-tile (one PSUM bank of f32)
NT = N // NS       # 4 n-tiles
PREFETCH = 5       # batches of x in flight

_CACHE = {}


def _build():
    import concourse.bacc as bacc
    from concourse import mybir
    from concourse.tile import TileContext

    f32 = mybir.dt.float32
    bf16 = mybir.dt.bfloat16
    Tanh = mybir.ActivationFunctionType.Tanh
    Exp = mybir.ActivationFunctionType.Exp

    nc = bacc.Bacc()
    xs = nc.declare_dram_parameter("xs", [BPC, H, N], f32, isOutput=False)
    xd = nc.declare_dram_parameter("xd", [BPC, H, N], f32, isOutput=False)
    # wt[k, h] = W[h, k] for k in [0, 512): rows 0:256 static, 256:512 dynamic
    wt = nc.declare_dram_parameter("wt", [2 * H, H], bf16, isOutput=False)
    # cb[h, b] = sum_k W[h, 512+k] * dec[b, k]  (host-precomputed bias)
    cb = nc.declare_dram_parameter("cb", [H, BPC], f32, isOutput=False)
    # vm[p, b, m, j] = v[m*128 + p] * (j == b); columns 16..31 are zero
    vm = nc.declare_dram_parameter("vm", [P, BPC, MT, 32], bf16, isOutput=False)
    # msk[p, j] = ((p%32) == (j%32)) & ((p%32) < 16): partition-sum matrix
    msk = nc.declare_dram_parameter("msk", [P, P], f32, isOutput=False)
    out = nc.declare_dram_parameter("out", [BPC, N], f32, isOutput=True)

    with (
        TileContext(nc) as tc,
        tc.tile_pool(name="const", bufs=1) as cpool,
        tc.tile_pool(name="xh6", bufs=PREFETCH + 1) as hpool6,
        tc.tile_pool(name="xh", bufs=PREFETCH) as hpool,
        tc.tile_pool(name="sc", bufs=2) as spool,
        tc.tile_pool(name="ps", bufs=6, space="PSUM") as ppool,
        tc.tile_pool(name="pl", bufs=1, space="PSUM") as plpool,
        tc.tile_pool(name="pq", bufs=1, space="PSUM") as pqpool,
    ):
        # --- x loads: one [128, 2, 2048] f32 tile per (batch, source);
        # partition p holds source rows p (k=0) and 128+p (k=1), each an
        # 8KB contiguous DRAM run. split>1 chops the column axis so the
        # first matmuls of batch 0/1 can start before the full 2MB lands.
        xf_tiles = {}

        def issue_x_dmas(bb, split=1):
            # flat [128, 2048] f32 tiles: one 8KB contiguous DRAM run per
            # partition => 128 cheap HWDGE descriptors per 1MB DMA (a
            # 2-runs-per-partition AP costs ~4x more descriptor-gen time
            # on the sync sequencer, measured 4.5-6.4us per 2MB DMA).
            # xs tiles get one extra slot so their DMAs for batch b+5 are
            # never slot-gated: half the bytes enter the ring a batch early,
            # keeping it from draining dry between slot-release bursts.
            tiles = [
                (hpool6 if pfx == "xsf" else hpool).tile(
                    [P, N], f32, name=f"{pfx}{k}_{bb}", tag=f"{pfx}{k}")
                for pfx in ("xsf", "xdf") for k in range(2)
            ]
            step = N // split
            for s in range(split):
                cs = slice(s * step, (s + 1) * step)
                for i, src_t in enumerate((xs, xs, xd, xd)):
                    k = i % 2
                    nc.sync.dma_start(
                        out=tiles[i][:, cs],
                        in_=src_t[bb, k * P:(k + 1) * P, cs],
                    )
            xf_tiles[bb] = tiles

        # --- replicated parameters on the idle GPSIMD (SWDGE) ring ---
        wt_sb = []
        for kt in range(KT):
            w = cpool.tile([P, H], bf16, name=f"wt{kt}", tag=f"wt{kt}")
            nc.gpsimd.dma_start(out=w[:], in_=wt[kt * P:(kt + 1) * P, :])
            wt_sb.append(w)
        vm_sb = cpool.tile([P, BPC, MT, 32], bf16)
        nc.gpsimd.dma_start(out=vm_sb[:], in_=vm[:])
        # bias laid out [128, m, b]
        c_sb = cpool.tile([P, MT, BPC], f32)
        nc.gpsimd.dma_start(out=c_sb[:], in_=cb[:].rearrange("(m p) b -> p m b", p=P))
        msk_sb = cpool.tile([P, P], f32)
        nc.gpsimd.dma_start(out=msk_sb[:], in_=msk[:])

        issue_x_dmas(0, split=4)
        issue_x_dmas(1, split=2)
        for bb in range(2, PREFETCH):
            issue_x_dmas(bb)

        # logits accumulator: ONE PSUM bank. The masked v-matmul for
        # (batch b, n-tile nt) lands batch b's 512 logits on partition
        # 32*nt + b, accumulating all 16 batches x 2 m-halves per quarter.
        lp = plpool.tile([P, NS], f32)

        sc_hist = {}

        def emit_vmms(vb):
            sc_prev = sc_hist.pop(vb)
            for m in range(MT):
                for nt in range(NT):
                    nc.tensor.matmul(
                        lp[32 * nt:32 * nt + 32, :],
                        lhsT=vm_sb[:, vb, m, :],
                        rhs=sc_prev[:, m, nt * NS:(nt + 1) * NS],
                        start=(vb == 0 and m == 0),
                        stop=(vb == BPC - 1 and m == MT - 1),
                        tile_position=(0, 32 * nt),
                    )

        # --- main loop ---
        for b in range(BPC):
            if b + PREFETCH < BPC:
                issue_x_dmas(b + PREFETCH)
            xh = [t[:].bitcast(bf16)[:, 1::2] for t in xf_tiles.pop(b)]

            # Two groups of 4 (nt, m) PSUM units, kt-outer WITHIN a group:
            # the batch's first 12 matmuls touch only the first three
            # tiles, so the last-arriving tile (kt3) plus its ~1.3-2us DMA
            # completion receipt is hidden behind ~3us of compute. Units
            # complete two per ~1.8us, keeping tanh/slot releases smooth.
            sc_t = spool.tile([P, MT, N], bf16, tag="sc")
            units = [(m, nt) for nt in range(NT) for m in range(MT)]
            for g in range(2):
                grp = units[4 * g:4 * g + 4]
                psts = {}
                for m, nt in grp:
                    psts[(m, nt)] = ppool.tile(
                        [P, NS], f32, tag="pst", name=f"pst{m}_{nt}"
                    )
                for kt in range(KT):
                    for m, nt in grp:
                        nc.tensor.matmul(
                            psts[(m, nt)][:],
                            lhsT=wt_sb[kt][:, m * P:(m + 1) * P],
                            rhs=xh[kt][:, nt * NS:(nt + 1) * NS],
                            start=(kt == 0),
                            stop=(kt == KT - 1),
                        )
                for m, nt in grp:
                    nc.scalar.activation(
                        sc_t[:, m, nt * NS:(nt + 1) * NS], psts[(m, nt)][:], Tanh,
                        bias=c_sb[:, m, b:b + 1],
                    )
            sc_hist[b] = sc_t
            if b > 0:
                emit_vmms(b - 1)
        emit_vmms(BPC - 1)

        # --- softmax over N per batch row (no max-subtraction: |logits| <~ 10)
        # exp of the whole logits bank at once; row sums via accum_out.
        exp_sb = cpool.tile([P, NS], f32)
        psums = cpool.tile([P, 1], f32)
        nc.scalar.activation(exp_sb[:], lp[:], Exp, accum_out=psums[:])
        # sum each batch's 4 quarter-sums across partitions: lpsum[32nt+b]
        # = sum_nt' psums[32nt'+b] for all 4 nt (f32 matmul, 1 column).
        lpsum = pqpool.tile([P, 1], f32)
        nc.tensor.matmul(lpsum[:], lhsT=msk_sb[:], rhs=psums[:], start=True, stop=True)
        rec = cpool.tile([P, 1], f32)
        nc.vector.reciprocal(rec[:], lpsum[:])
        nc.vector.tensor_scalar_mul(exp_sb[:], exp_sb[:], rec[:])
        for nt in range(NT):
            nc.sync.dma_start(
                out=out[:, nt * NS:(nt + 1) * NS],
                in_=exp_sb[32 * nt:32 * nt + BPC, :],
            )

    nc.compile()
    return nc


def _make_in_maps(static_hidden, dynamic_hidden, decoder_hidden, v, W):
    import ml_dtypes

    bf16 = ml_dtypes.bfloat16
    W0 = np.asarray(W, dtype=np.float32)[0]          # [256, 768]
    wt_np = np.ascontiguousarray(W0[:, :2 * H].T.astype(bf16))   # [512, 256]
    vhalf = np.asarray(v, dtype=np.float32)[0, 0].reshape(MT, P)  # [2, 128]
    # vm[p, b, m, j] = v[m*128+p] * (j == b); j in [0, 32), cols 16..31 zero
    eye = np.zeros((BPC, 32), dtype=np.float32)
    eye[np.arange(BPC), np.arange(BPC)] = 1.0
    vm_np = np.ascontiguousarray(
        np.einsum("mp,bj->pbmj", vhalf, eye).astype(bf16)
    )
    # msk[p, j] = ((p%32) == (j%32)) & ((p%32) < 16)
    pp = np.arange(P)
    msk_np = np.ascontiguousarray(
        (((pp[:, None] % 32) == (pp[None, :] % 32)) & ((pp[:, None] % 32) < 16))
        .astype(np.float32)
    )

    sh = np.asarray(static_hidden, dtype=np.float32)
    dh = np.asarray(dynamic_hidden, dtype=np.float32)
    dec = np.asarray(decoder_hidden, dtype=np.float32)
    # cb[h, b] = sum_k W_dec[h, k] dec[b, k], fp32 on host (tiny)
    cb_full = W0[:, 2 * H:] @ dec.T                  # [256, B]

    in_maps = []
    for i in range(NCORES):
        sl = slice(i * BPC, (i + 1) * BPC)
        in_maps.append({
            "xs": np.ascontiguousarray(sh[sl]),
            "xd": np.ascontiguousarray(dh[sl]),
            "wt": wt_np,
            "cb": np.ascontiguousarray(cb_full[:, sl]),
            "vm": vm_np,
            "msk": msk_np,
        })
    return in_maps


def kernel(static_hidden, dynamic_hidden, decoder_hidden, v, W):
    from concourse.bass_utils import run_bass_kernel_spmd

    if "nc" not in _CACHE:
        _CACHE["nc"] = _build()
    nc = _CACHE["nc"]

    in_maps = _make_in_maps(static_hidden, dynamic_hidden, decoder_hidden, v, W)
    res = run_bass_kernel_spmd(nc, in_maps, core_ids=list(range(NCORES)))
    out = np.concatenate([r["out"] for r in res.results], axis=0)
    return out.reshape(B, 1, N).astype(np.float32)


# revision 12
# speedup vs baseline: 1.0265x; 1.0265x over previous
"""Bahdanau-style attention scoring kernel for 8 TRN2 NeuronCores.

Reference computation (B=128, H=256, N=2048):
    hidden = concat([static, dynamic, broadcast(dec)], axis=1)   # [B, 3H, N]
    scores = tanh(einsum('hk,bkn->bhn', W[0], hidden))           # [B, H, N]
    logits = einsum('h,bhn->bn', v[0,0], scores)                 # [B, N]
    attns  = softmax(logits, axis=-1)[:, None, :]                # [B, 1, N]

Strategy v2 (v1 measured ~210-230us; this version targets ~175us):

- Data-parallel over batch: 16 batches per core, no collectives. The tiny
  W / v params are replicated (pre-cast to bf16 on host); the broadcast
  decoder term collapses to a per-batch bias c[b] = W_dec @ dec[b]
  (precomputed on host, 0.003% of FLOPs).

- Matmuls run in bf16 at ~222ns per 512-col tile (near the 2.4GHz PE
  roofline). f32 activations are DMA'd raw; the matmul rhs reads the high
  2 bytes of each f32 in SBUF (stride-2 bf16 bitcast view) -- bf16
  truncation for free. PE work per batch ~8.9us (32 mains + 8 v-matmuls);
  DMA per batch 4MB ~9.5us at the measured ~420GB/s => DMA-bound steady
  state with the PE never the gating leg.

- v1's trace showed a 13.5us/batch period: the PE stalled ~3.5us/batch on
  a tanh->PSUM-slot WAR chain (2x2-bank pst slots) plus ~1us on late x
  tiles. v2 uses 1-bank PSUM accumulators with 6 slots (tag "pst"), so
  slot reuse lands ~6 tanh-units back and the chain has ~4us of slack.

- All 64 logits land in ONE PSUM bank: the masked v-matmul for n-tile nt
  writes output partitions [32*nt, 32*nt+32) (tile_position col offsets
  must be multiples of 32), with vm columns 16..31 zero so the unused
  partitions accumulate exact zeros instead of stale PSUM garbage.
  Softmax is then: one exp over [128,512] with accum_out row sums, a tiny
  f32 mask-matmul that sums each batch's 4 quarter-sums across partitions
  (placing the total on all 4 of that batch's partitions), reciprocal,
  one [128,512] scale, 4 output DMAs.

- x loads: one 2MB HWDGE DMA per (batch, source) (split into column
  chunks for batches 0/1 so the first matmuls start ~4us in), prefetched
  5 batches deep. Parameter DMAs go on the idle GPSIMD (SWDGE) ring so
  the first x tile is not queued behind them.

Built as a bacc.Bacc graph (its compile() pass redistributes multi-sem
waits; raw Bass hits the hardware's one-sync-wait-per-instruction limit).
"""

import sys

if "/opt/trn_rl_repo" not in sys.path:
    sys.path.insert(0, "/opt/trn_rl_repo")

import numpy as np

B, H, N = 128, 256, 2048
NCORES = 8
BPC = B // NCORES  # batches per core
P = 128            # SBUF partitions
KT = 4             # k-tiles over 2H=512 contraction
MT = 2             # m-tiles over H=256 output rows
NS = 512           # n-tile (one PSUM bank of f32)
NT = N // NS       # 4 n-tiles
PREFETCH = 5       # batches of x in flight

_CACHE = {}


def _build():
    import concourse.bacc as bacc
    from concourse import mybir
    from concourse.tile import TileContext

    f32 = mybir.dt.float32
    bf16 = mybir.dt.bfloat16
    Tanh = mybir.ActivationFunctionType.Tanh
    Exp = mybir.ActivationFunctionType.Exp

    nc = bacc.Bacc()
    xs = nc.declare_dram_parameter("xs", [BPC, H, N], f32, isOutput=False)
    xd = nc.declare_dram_parameter("xd", [BPC, H, N], f32, isOutput=False)
    # wt[k, h] = W[h, k] for k in [0, 512): rows 0:256 static, 256:512 dynamic
    wt = nc.declare_dram_parameter("wt", [2 * H, H], bf16, isOutput=False)
    # cb[h, b] = sum_k W[h, 512+k] * dec[b, k]  (host-precomputed bias)
    cb = nc.declare_dram_parameter("cb", [H, BPC], f32, isOutput=False)
    # vm[p, b, m, j] = v[m*128 + p] * (j == b); columns 16..31 are zero
    vm = nc.declare_dram_parameter("vm", [P, BPC, MT, 32], bf16, isOutput=False)
    # msk[p, j] = ((p%32) == (j%32)) & ((p%32) < 16): partition-sum matrix
    msk = nc.declare_dram_parameter("msk", [P, P], f32, isOutput=False)
    out = nc.declare_dram_parameter("out", [BPC, N], f32, isOutput=True)

    with (
        TileContext(nc) as tc,
        tc.tile_pool(name="const", bufs=1) as cpool,
        tc.tile_pool(name="xh", bufs=PREFETCH) as hpool,
        tc.tile_pool(name="sc", bufs=2) as spool,
        tc.tile_pool(name="ps", bufs=6, space="PSUM") as ppool,
        tc.tile_pool(name="pl", bufs=1, space="PSUM") as plpool,
        tc.tile_pool(name="pq", bufs=1, space="PSUM") as pqpool,
    ):
        # --- x loads: one [128, 2, 2048] f32 tile per (batch, source);
        # partition p holds source rows p (k=0) and 128+p (k=1), each an
        # 8KB contiguous DRAM run. split>1 chops the column axis so the
        # first matmuls of batch 0/1 can start before the full 2MB lands.
        xf_tiles = {}

        def issue_x_dmas(bb, split=1):
            # flat [128, 2048] f32 tiles: one 8KB contiguous DRAM run per
            # partition => 128 cheap HWDGE descriptors per 1MB DMA (a
            # 2-runs-per-partition AP costs ~4x more descriptor-gen time
            # on the sync sequencer, measured 4.5-6.4us per 2MB DMA).
            tiles = [
                hpool.tile([P, N], f32, name=f"{pfx}{k}_{bb}", tag=f"{pfx}{k}")
                for pfx in ("xsf", "xdf") for k in range(2)
            ]
            step = N // split
            for s in range(split):
                cs = slice(s * step, (s + 1) * step)
                for i, src_t in enumerate((xs, xs, xd, xd)):
                    k = i % 2
                    nc.sync.dma_start(
                        out=tiles[i][:, cs],
                        in_=src_t[bb, k * P:(k + 1) * P, cs],
                    )
            xf_tiles[bb] = tiles

        # --- replicated parameters on the idle GPSIMD (SWDGE) ring ---
        wt_sb = []
        for kt in range(KT):
            w = cpool.tile([P, H], bf16, name=f"wt{kt}", tag=f"wt{kt}")
            nc.gpsimd.dma_start(out=w[:], in_=wt[kt * P:(kt + 1) * P, :])
            wt_sb.append(w)
        vm_sb = cpool.tile([P, BPC, MT, 32], bf16)
        nc.gpsimd.dma_start(out=vm_sb[:], in_=vm[:])
        # bias laid out [128, m, b]
        c_sb = cpool.tile([P, MT, BPC], f32)
        nc.gpsimd.dma_start(out=c_sb[:], in_=cb[:].rearrange("(m p) b -> p m b", p=P))
        msk_sb = cpool.tile([P, P], f32)
        nc.gpsimd.dma_start(out=msk_sb[:], in_=msk[:])

        issue_x_dmas(0, split=4)
        issue_x_dmas(1, split=2)
        for bb in range(2, PREFETCH):
            issue_x_dmas(bb)

        # logits accumulator: ONE PSUM bank. The masked v-matmul for
        # (batch b, n-tile nt) lands batch b's 512 logits on partition
        # 32*nt + b, accumulating all 16 batches x 2 m-halves per quarter.
        lp = plpool.tile([P, NS], f32)

        sc_hist = {}

        def emit_vmms(vb):
            sc_prev = sc_hist.pop(vb)
            for m in range(MT):
                for nt in range(NT):
                    nc.tensor.matmul(
                        lp[32 * nt:32 * nt + 32, :],
                        lhsT=vm_sb[:, vb, m, :],
                        rhs=sc_prev[:, m, nt * NS:(nt + 1) * NS],
                        start=(vb == 0 and m == 0),
                        stop=(vb == BPC - 1 and m == MT - 1),
                        tile_position=(0, 32 * nt),
                    )

        # --- main loop ---
        for b in range(BPC):
            if b + PREFETCH < BPC:
                issue_x_dmas(b + PREFETCH)
            xh = [t[:].bitcast(bf16)[:, 1::2] for t in xf_tiles.pop(b)]

            # nt-outer / kt-inner: each (nt, m) PSUM unit's 4 accumulating
            # matmuls run consecutively and its tanh follows immediately,
            # so slot releases (and the sc columns the v-matmuls need) are
            # produced evenly through the batch instead of bunching at
            # m-group ends.
            sc_t = spool.tile([P, MT, N], bf16, tag="sc")
            for nt in range(NT):
                ns = slice(nt * NS, (nt + 1) * NS)
                for m in range(MT):
                    pst = ppool.tile([P, NS], f32, tag="pst", name=f"pst{m}_{nt}")
                    for kt in range(KT):
                        nc.tensor.matmul(
                            pst[:],
                            lhsT=wt_sb[kt][:, m * P:(m + 1) * P],
                            rhs=xh[kt][:, ns],
                            start=(kt == 0),
                            stop=(kt == KT - 1),
                        )
                    nc.scalar.activation(
                        sc_t[:, m, ns], pst[:], Tanh,
                        bias=c_sb[:, m, b:b + 1],
                    )
            sc_hist[b] = sc_t
            if b > 0:
                emit_vmms(b - 1)
        emit_vmms(BPC - 1)

        # --- softmax over N per batch row (no max-subtraction: |logits| <~ 10)
        # exp of the whole logits bank at once; row sums via accum_out.
        exp_sb = cpool.tile([P, NS], f32)
        psums = cpool.tile([P, 1], f32)
        nc.scalar.activation(exp_sb[:], lp[:], Exp, accum_out=psums[:])
        # sum each batch's 4 quarter-sums across partitions: lpsum[32nt+b]
        # = sum_nt' psums[32nt'+b] for all 4 nt (f32 matmul, 1 column).
        lpsum = pqpool.tile([P, 1], f32)
        nc.tensor.matmul(lpsum[:], lhsT=msk_sb[:], rhs=psums[:], start=True, stop=True)
        rec = cpool.tile([P, 1], f32)
        nc.vector.reciprocal(rec[:], lpsum[:])
        nc.vector.tensor_scalar_mul(exp_sb[:], exp_sb[:], rec[:])
        for nt in range(NT):
            nc.sync.dma_start(
                out=out[:, nt * NS:(nt + 1) * NS],
                in_=exp_sb[32 * nt:32 * nt + BPC, :],
            )

    nc.compile()
    return nc


def _make_in_maps(static_hidden, dynamic_hidden, decoder_hidden, v, W):
    import ml_dtypes

    bf16 = ml_dtypes.bfloat16
    W0 = np.asarray(W, dtype=np.float32)[0]          # [256, 768]
    wt_np = np.ascontiguousarray(W0[:, :2 * H].T.astype(bf16))   # [512, 256]
    vhalf = np.asarray(v, dtype=np.float32)[0, 0].reshape(MT, P)  # [2, 128]
    # vm[p, b, m, j] = v[m*128+p] * (j == b); j in [0, 32), cols 16..31 zero
    eye = np.zeros((BPC, 32), dtype=np.float32)
    eye[np.arange(BPC), np.arange(BPC)] = 1.0
    vm_np = np.ascontiguousarray(
        np.einsum("mp,bj->pbmj", vhalf, eye).astype(bf16)
    )
    # msk[p, j] = ((p%32) == (j%32)) & ((p%32) < 16)
    pp = np.arange(P)
    msk_np = np.ascontiguousarray(
        (((pp[:, None] % 32) == (pp[None, :] % 32)) & ((pp[:, None] % 32) < 16))
        .astype(np.float32)
    )

    sh = np.asarray(static_hidden, dtype=np.float32)
    dh = np.asarray(dynamic_hidden, dtype=np.float32)
    dec = np.asarray(decoder_hidden, dtype=np.float32)
    # cb[h, b] = sum_k W_dec[h, k] dec[b, k], fp32 on host (tiny)
    cb_full = W0[:, 2 * H:] @ dec.T                  # [256, B]

    in_maps = []
    for i in range(NCORES):
        sl = slice(i * BPC, (i + 1) * BPC)
        in_maps.append({
            "xs": np.ascontiguousarray(sh[sl]),
            "xd": np.ascontiguousarray(dh[sl]),
            "wt": wt_np,
            "cb": np.ascontiguousarray(cb_full[:, sl]),
            "vm": vm_np,
            "msk": msk_np,
        })
    return in_maps


def kernel(static_hidden, dynamic_hidden, decoder_hidden, v, W):
    from concourse.bass_utils import run_bass_kernel_spmd

    if "nc" not in _CACHE:
        _CACHE["nc"] = _build()
    nc = _CACHE["nc"]

    in_maps = _make_in_maps(static_hidden, dynamic_hidden, decoder_hidden, v, W)
    res = run_bass_kernel_spmd(nc, in_maps, core_ids=list(range(NCORES)))
    out = np.concatenate([r["out"] for r in res.results], axis=0)
    return out.reshape(B, 1, N).astype(np.float32)


# revision 13
# speedup vs baseline: 1.1350x; 1.1057x over previous
"""Bahdanau-style attention scoring kernel for 8 TRN2 NeuronCores.

Reference computation (B=128, H=256, N=2048):
    hidden = concat([static, dynamic, broadcast(dec)], axis=1)   # [B, 3H, N]
    scores = tanh(einsum('hk,bkn->bhn', W[0], hidden))           # [B, H, N]
    logits = einsum('h,bhn->bn', v[0,0], scores)                 # [B, N]
    attns  = softmax(logits, axis=-1)[:, None, :]                # [B, 1, N]

Strategy (measured 195-200us on a quiet chip, 215-235us when the chip's
HBM throughput sags to ~320GB/s mid-run -- that mode is environmental,
identical NEFFs measure both ways; earlier versions were 210-290us):

- Data-parallel over batch: 16 batches per core, no collectives. The tiny
  W / v params are replicated (pre-cast to bf16 on host); the broadcast
  decoder term collapses to a per-batch bias c[b] = W_dec @ dec[b]
  (precomputed on host, 0.003% of FLOPs).

- Matmuls run in bf16 at ~227ns per 512-col tile (near the 2.4GHz PE
  roofline). f32 activations are DMA'd raw; the matmul rhs reads the high
  2 bytes of each f32 in SBUF (stride-2 bf16 bitcast view) -- bf16
  truncation for free, no cast pass on any engine. End-to-end rel err
  ~2e-3 vs the 2e-2 gate.

- x loads: four flat 1MB HWDGE DMAs per batch ([128, 2048] f32, one 8KB
  contiguous DRAM run per partition = 128 cheap descriptors), prefetched
  5 batches deep. The ring sustains ~420GB/s; 1MB is the sweet spot
  (512KB DMAs dropped the ring to ~310, and 2MB two-runs-per-partition
  APs cost 4.5-6.4us of HWDGE descriptor-gen each, saturating the sync
  sequencer). Batches 0/1 are column-split (4/2 chunks, chunk-major
  issue order) so the first matmuls start ~11us in. Parameter DMAs go
  on the idle GPSIMD (SWDGE) ring.

- Main loop is nt-outer / kt-inner over 1-bank PSUM units with 6 slots:
  each (nt, m) unit's 4 accumulating matmuls run consecutively and its
  tanh follows immediately, so tanh work, PSUM slot releases, and the sc
  columns the v-matmuls need are produced evenly through the batch. (The
  v1 kernel's 2x2-bank slots stalled the PE ~3.5us/batch on the
  tanh->slot-WAR chain; this shape has ~4us of slack on it.)

- All 64 logits land in ONE PSUM bank: the masked v-matmul for n-tile nt
  writes output partitions [32*nt, 32*nt+32) via an explicit
  tile_position=(0, 32*nt) (the auto-inferred path rejects base 96), with
  vm columns 16..31 zero so the unused partitions accumulate exact zeros
  instead of stale PSUM garbage. v-matmuls run one batch behind the
  mains. Softmax is then: one exp over the whole [128,512] bank with
  accum_out row sums, a tiny f32 mask-matmul that sums each batch's 4
  quarter-sums across partitions, reciprocal, one [128,512] scale, and 4
  output DMAs -- ~6us of tail.

Built as a bacc.Bacc graph (its compile() pass redistributes multi-sem
waits; raw Bass hits the hardware's one-sync-wait-per-instruction limit).
"""

import sys

if "/opt/trn_rl_repo" not in sys.path:
    sys.path.insert(0, "/opt/trn_rl_repo")

import numpy as np

B, H, N = 128, 256, 2048
NCORES = 8
BPC = B // NCORES  # batches per core
P = 128            # SBUF partitions
KT = 4             # k-tiles over 2H=512 contraction
MT = 2             # m-tiles over H=256 output rows
NS = 512           # n-tile (one PSUM bank of f32)
NT = N // NS       # 4 n-tiles
PREFETCH = 5       # batches of x in flight

_CACHE = {}


def _build():
    import concourse.bacc as bacc
    from concourse import mybir
    from concourse.tile import TileContext

    f32 = mybir.dt.float32
    bf16 = mybir.dt.bfloat16
    Tanh = mybir.ActivationFunctionType.Tanh
    Exp = mybir.ActivationFunctionType.Exp

    nc = bacc.Bacc()
    xs = nc.declare_dram_parameter("xs", [BPC, H, N], f32, isOutput=False)
    xd = nc.declare_dram_parameter("xd", [BPC, H, N], f32, isOutput=False)
    # wt[k, h] = W[h, k] for k in [0, 512): rows 0:256 static, 256:512 dynamic
    wt = nc.declare_dram_parameter("wt", [2 * H, H], bf16, isOutput=False)
    # cb[h, b] = sum_k W[h, 512+k] * dec[b, k]  (host-precomputed bias)
    cb = nc.declare_dram_parameter("cb", [H, BPC], f32, isOutput=False)
    # vm[p, b, m, j] = v[m*128 + p] * (j == b); columns 16..31 are zero
    vm = nc.declare_dram_parameter("vm", [P, BPC, MT, 32], bf16, isOutput=False)
    # msk[p, j] = ((p%32) == (j%32)) & ((p%32) < 16): partition-sum matrix
    msk = nc.declare_dram_parameter("msk", [P, P], f32, isOutput=False)
    out = nc.declare_dram_parameter("out", [BPC, N], f32, isOutput=True)

    with (
        TileContext(nc) as tc,
        tc.tile_pool(name="const", bufs=1) as cpool,
        tc.tile_pool(name="xh", bufs=PREFETCH) as hpool,
        tc.tile_pool(name="sc", bufs=2) as spool,
        tc.tile_pool(name="ps", bufs=6, space="PSUM") as ppool,
        tc.tile_pool(name="pl", bufs=1, space="PSUM") as plpool,
        tc.tile_pool(name="pq", bufs=1, space="PSUM") as pqpool,
    ):
        # --- x loads: one [128, 2, 2048] f32 tile per (batch, source);
        # partition p holds source rows p (k=0) and 128+p (k=1), each an
        # 8KB contiguous DRAM run. split>1 chops the column axis so the
        # first matmuls of batch 0/1 can start before the full 2MB lands.
        xf_tiles = {}

        def issue_x_dmas(bb, split=1):
            # flat [128, 2048] f32 tiles: one 8KB contiguous DRAM run per
            # partition => 128 cheap HWDGE descriptors per 1MB DMA (a
            # 2-runs-per-partition AP costs ~4x more descriptor-gen time
            # on the sync sequencer, measured 4.5-6.4us per 2MB DMA).
            tiles = [
                hpool.tile([P, N], f32, name=f"{pfx}{k}_{bb}", tag=f"{pfx}{k}")
                for pfx in ("xsf", "xdf") for k in range(2)
            ]
            step = N // split
            for s in range(split):
                cs = slice(s * step, (s + 1) * step)
                for i, src_t in enumerate((xs, xs, xd, xd)):
                    k = i % 2
                    nc.sync.dma_start(
                        out=tiles[i][:, cs],
                        in_=src_t[bb, k * P:(k + 1) * P, cs],
                    )
            xf_tiles[bb] = tiles

        # --- replicated parameters on the idle GPSIMD (SWDGE) ring ---
        wt_sb = []
        for kt in range(KT):
            w = cpool.tile([P, H], bf16, name=f"wt{kt}", tag=f"wt{kt}")
            nc.gpsimd.dma_start(out=w[:], in_=wt[kt * P:(kt + 1) * P, :])
            wt_sb.append(w)
        vm_sb = cpool.tile([P, BPC, MT, 32], bf16)
        nc.gpsimd.dma_start(out=vm_sb[:], in_=vm[:])
        # bias laid out [128, m, b]
        c_sb = cpool.tile([P, MT, BPC], f32)
        nc.gpsimd.dma_start(out=c_sb[:], in_=cb[:].rearrange("(m p) b -> p m b", p=P))
        msk_sb = cpool.tile([P, P], f32)
        nc.gpsimd.dma_start(out=msk_sb[:], in_=msk[:])

        issue_x_dmas(0, split=4)
        issue_x_dmas(1, split=2)
        for bb in range(2, PREFETCH):
            issue_x_dmas(bb)

        # logits accumulator: ONE PSUM bank. The masked v-matmul for
        # (batch b, n-tile nt) lands batch b's 512 logits on partition
        # 32*nt + b, accumulating all 16 batches x 2 m-halves per quarter.
        lp = plpool.tile([P, NS], f32)

        sc_hist = {}

        def emit_vmms(vb):
            sc_prev = sc_hist.pop(vb)
            for m in range(MT):
                for nt in range(NT):
                    nc.tensor.matmul(
                        lp[32 * nt:32 * nt + 32, :],
                        lhsT=vm_sb[:, vb, m, :],
                        rhs=sc_prev[:, m, nt * NS:(nt + 1) * NS],
                        start=(vb == 0 and m == 0),
                        stop=(vb == BPC - 1 and m == MT - 1),
                        tile_position=(0, 32 * nt),
                    )

        # --- main loop ---
        for b in range(BPC):
            if b + PREFETCH < BPC:
                issue_x_dmas(b + PREFETCH)
            xh = [t[:].bitcast(bf16)[:, 1::2] for t in xf_tiles.pop(b)]

            # nt-outer / kt-inner: each (nt, m) PSUM unit's 4 accumulating
            # matmuls run consecutively and its tanh follows immediately,
            # so slot releases (and the sc columns the v-matmuls need) are
            # produced evenly through the batch instead of bunching at
            # m-group ends.
            sc_t = spool.tile([P, MT, N], bf16, tag="sc")
            for nt in range(NT):
                ns = slice(nt * NS, (nt + 1) * NS)
                for m in range(MT):
                    pst = ppool.tile([P, NS], f32, tag="pst", name=f"pst{m}_{nt}")
                    for kt in range(KT):
                        nc.tensor.matmul(
                            pst[:],
                            lhsT=wt_sb[kt][:, m * P:(m + 1) * P],
                            rhs=xh[kt][:, ns],
                            start=(kt == 0),
                            stop=(kt == KT - 1),
                        )
                    nc.scalar.activation(
                        sc_t[:, m, ns], pst[:], Tanh,
                        bias=c_sb[:, m, b:b + 1],
                    )
            sc_hist[b] = sc_t
            if b > 0:
                emit_vmms(b - 1)
        emit_vmms(BPC - 1)

        # --- softmax over N per batch row (no max-subtraction: |logits| <~ 10)
        # exp of the whole logits bank at once; row sums via accum_out.
        exp_sb = cpool.tile([P, NS], f32)
        psums = cpool.tile([P, 1], f32)
        nc.scalar.activation(exp_sb[:], lp[:], Exp, accum_out=psums[:])
        # sum each batch's 4 quarter-sums across partitions: lpsum[32nt+b]
        # = sum_nt' psums[32nt'+b] for all 4 nt (f32 matmul, 1 column).
        lpsum = pqpool.tile([P, 1], f32)
        nc.tensor.matmul(lpsum[:], lhsT=msk_sb[:], rhs=psums[:], start=True, stop=True)
        rec = cpool.tile([P, 1], f32)
        nc.vector.reciprocal(rec[:], lpsum[:])
        nc.vector.tensor_scalar_mul(exp_sb[:], exp_sb[:], rec[:])
        for nt in range(NT):
            nc.sync.dma_start(
                out=out[:, nt * NS:(nt + 1) * NS],
                in_=exp_sb[32 * nt:32 * nt + BPC, :],
            )

    nc.compile()
    return nc


def _make_in_maps(static_hidden, dynamic_hidden, decoder_hidden, v, W):
    import ml_dtypes

    bf16 = ml_dtypes.bfloat16
    W0 = np.asarray(W, dtype=np.float32)[0]          # [256, 768]
    wt_np = np.ascontiguousarray(W0[:, :2 * H].T.astype(bf16))   # [512, 256]
    vhalf = np.asarray(v, dtype=np.float32)[0, 0].reshape(MT, P)  # [2, 128]
    # vm[p, b, m, j] = v[m*128+p] * (j == b); j in [0, 32), cols 16..31 zero
    eye = np.zeros((BPC, 32), dtype=np.float32)
    eye[np.arange(BPC), np.arange(BPC)] = 1.0
    vm_np = np.ascontiguousarray(
        np.einsum("mp,bj->pbmj", vhalf, eye).astype(bf16)
    )
    # msk[p, j] = ((p%32) == (j%32)) & ((p%32) < 16)
    pp = np.arange(P)
    msk_np = np.ascontiguousarray(
        (((pp[:, None] % 32) == (pp[None, :] % 32)) & ((pp[:, None] % 32) < 16))
        .astype(np.float32)
    )

    sh = np.asarray(static_hidden, dtype=np.float32)
    dh = np.asarray(dynamic_hidden, dtype=np.float32)
    dec = np.asarray(decoder_hidden, dtype=np.float32)
    # cb[h, b] = sum_k W_dec[h, k] dec[b, k], fp32 on host (tiny)
    cb_full = W0[:, 2 * H:] @ dec.T                  # [256, B]

    in_maps = []
    for i in range(NCORES):
        sl = slice(i * BPC, (i + 1) * BPC)
        in_maps.append({
            "xs": np.ascontiguousarray(sh[sl]),
            "xd": np.ascontiguousarray(dh[sl]),
            "wt": wt_np,
            "cb": np.ascontiguousarray(cb_full[:, sl]),
            "vm": vm_np,
            "msk": msk_np,
        })
    return in_maps


def kernel(static_hidden, dynamic_hidden, decoder_hidden, v, W):
    from concourse.bass_utils import run_bass_kernel_spmd

    if "nc" not in _CACHE:
        _CACHE["nc"] = _build()
    nc = _CACHE["nc"]

    in_maps = _make_in_maps(static_hidden, dynamic_hidden, decoder_hidden, v, W)
    res = run_bass_kernel_spmd(nc, in_maps, core_ids=list(range(NCORES)))
    out = np.concatenate([r["out"] for r in res.results], axis=0)
    return out.reshape(B, 1, N).astype(np.float32)
